# revision 1
# baseline (speedup 1.0000x reference)
"""Trainium2 Bass kernel for nn_CustomDistanceLayer (variance-weighted distance
+ 32x32 stride-1 box-sum pooling).

Reference computation (shapes hardcoded):
    kernel = tile(input_image[32,32] -> [4096,4096])
    dist   = (kernel - som_matrix)^2 / (som_running_variances + 1e-8)
    out    = 32x32 valid box-sum of dist -> [4065, 4065]

Strategy (8 NeuronCores, SPMD, row-sharded with 31-row halo):
  * Every core runs the SAME program on a 543-row slab (512 output rows + 31
    halo rows); slab starts overlap slightly so all shapes are uniform.
  * Host passes a per-core pre-negated tiled-image block (handles slab start
    not being a multiple of 32 via a row roll).
  * Per core: diff = som + (-kern) on DVE, Square on ScalarE, 1/var via the
    DVE fast reciprocal, horizontal sliding box-sum in a single
    tensor_tensor_scan pass (h[j] = h[j-1] + d[j+31] - d[j-1], fp32 state,
    bf16 stream), vertical 32-row band-sum as two accumulating bf16 matmuls
    against banded 0/1 weights on the TensorEngine, ScalarE PSUM drain,
    DMA out split over the three DMA trigger paths (som on the SP HWDGE
    ring, var on SWDGE, outputs on the ACT HWDGE ring).  (A DMA-accumulate variant that adds som onto a
    prefilled -kern tile in the SDMA CCE exists behind use_accum=True, but it
    crashes the device under the axon/bass2jax path, so it stays off.)
"""
import numpy as np
import ml_dtypes

import concourse.bass as bass
import concourse.mybir as mybir
import concourse.tile as tile
from concourse import bacc
from concourse.bass_utils import run_bass_kernel_spmd

K = 32
HH = 4096
OUT = HH - K + 1  # 4065
N_CORES = 8
OUT_ROWS = 512
DIST_ROWS = OUT_ROWS + K - 1  # 543
STARTS = [round(c * (OUT - OUT_ROWS) / (N_CORES - 1)) for c in range(N_CORES)]

PB = [128, 128, 128, 128, DIST_ROWS - 512]  # partition rows per block
RB = [0, 128, 256, 384, 512]
N_BLK = len(PB)
N_OB = 4  # output row-blocks of 128

F32 = mybir.dt.float32
BF16 = mybir.dt.bfloat16

# column chunks for the vertical matmul (PSUM free-dim limit 512 for f32 out)
JCHUNKS = [(j, min(512, OUT - j)) for j in range(0, OUT, 512)]

_PROGRAM_CACHE = {}


def _band_w1():
    k = np.arange(128)[:, None]
    m = np.arange(128)[None, :]
    return ((m <= k) & (k <= m + K - 1)).astype(ml_dtypes.bfloat16)


def _band_w2():
    kk = np.arange(K - 1)[:, None]
    m = np.arange(128)[None, :]
    return (m >= kk + 128 - (K - 1)).astype(ml_dtypes.bfloat16)


def build_program(use_accum=False, repeat=1, gps_ops=0, interleave=True):
    """gps_ops: how many of the two tensor-tensor passes run on GPSIMD
    (0: none, 1: the kern-add, 2: add + the sq*w multiply)."""
    nc = bacc.Bacc("TRN2", target_bir_lowering=False, debug=False)
    som = nc.dram_tensor("som", [DIST_ROWS, HH], F32, kind="ExternalInput").ap()
    var = nc.dram_tensor("var", [DIST_ROWS, HH], F32, kind="ExternalInput").ap()
    nkern = nc.dram_tensor("nkern", [128, HH], F32, kind="ExternalInput").ap()
    w1d = nc.dram_tensor("w1", [128, 128], BF16, kind="ExternalInput").ap()
    w2d = nc.dram_tensor("w2", [K - 1, 128], BF16, kind="ExternalInput").ap()
    out = nc.dram_tensor("out", [OUT_ROWS, OUT], F32, kind="ExternalOutput").ap()

    with tile.TileContext(nc) as tc:
        with (
            tc.tile_pool(name="const", bufs=1) as constp,
            tc.tile_pool(name="som", bufs=3) as somp,
            tc.tile_pool(name="var", bufs=2) as varp,
            tc.tile_pool(name="w", bufs=1) as wp,
            tc.tile_pool(name="d", bufs=2) as dp,
            tc.tile_pool(name="h0", bufs=2) as h0p,
            tc.tile_pool(name="h", bufs=3) as hp,
            tc.tile_pool(name="outp", bufs=2) as outp,
            tc.tile_pool(name="psum", bufs=8, space="PSUM") as psump,
        ):
            nkern_sb = constp.tile([128, HH], F32)
            nc.sync.dma_start(nkern_sb[:], nkern[:, :])
            w1_sb = constp.tile([128, 128], BF16)
            nc.sync.dma_start(w1_sb[:], w1d[:, :])
            w2_sb = constp.tile([K - 1, 128], BF16)
            nc.sync.dma_start(w2_sb[:], w2d[:, :])

            add_eng = nc.gpsimd if gps_ops >= 1 else nc.vector
            mul_eng = nc.gpsimd if gps_ops >= 2 else nc.vector

            for _ in range(repeat):
                h_blocks = []

                def emit_block(b):
                    p = PB[b]
                    rows = slice(RB[b], RB[b] + p)
                    som_t = somp.tile([p, HH], F32)
                    # som on the SP HWDGE ring, var on the ACT HWDGE ring,
                    # outputs on SWDGE: three DMA paths run in parallel
                    nc.sync.dma_start(som_t[:], som[rows, :])
                    # diff = som + (-kern), in place over som
                    add_eng.tensor_add(som_t[:], som_t[:], nkern_sb[:p, :])
                    var_t = varp.tile([p, HH], F32)
                    nc.gpsimd.dma_start(var_t[:], var[rows, :])
                    w_t = wp.tile([p, HH], F32)
                    nc.vector.reciprocal_approx_fast(w_t[:], var_t[:])
                    # sq = diff^2 in place (ScalarE)
                    nc.scalar.activation(
                        som_t[:], som_t[:], mybir.ActivationFunctionType.Square
                    )
                    # d = sq * w -> bf16 (16-bit scan input; scan state is fp32)
                    d_t = dp.tile([p, HH], BF16)
                    mul_eng.tensor_mul(d_t[:], som_t[:], w_t[:])
                    # sliding 32-wide window sum in ONE scan pass:
                    #   h[0] = sum(d[0:32]);  h[j] = h[j-1] + d[j+31] - d[j-1]
                    # (bf16 d errors cancel exactly when an element leaves the
                    # window; only fp32 state rounding accumulates)
                    h_t = hp.tile([p, OUT], BF16)
                    h0 = h0p.tile([p, 1], F32)
                    nc.vector.tensor_reduce(
                        h0[:], d_t[:, 0:K], mybir.AxisListType.X, mybir.AluOpType.add
                    )
                    nc.vector.tensor_copy(h_t[:, 0:1], h0[:])
                    nc.vector.tensor_tensor_scan(
                        h_t[:, 1:OUT],
                        d_t[:, K:HH],
                        d_t[:, 0 : OUT - 1],
                        initial=h0[:],
                        op0=mybir.AluOpType.add,
                        op1=mybir.AluOpType.subtract,
                    )
                    h_blocks.append(h_t)

                def emit_mm_group(ib):
                    out_t = outp.tile([128, OUT], F32)
                    psums = []
                    for j0, jw in JCHUNKS:
                        ps = psump.tile([128, jw], F32)
                        nc.tensor.matmul(
                            ps[:],
                            w1_sb[:],
                            h_blocks[ib][:, j0 : j0 + jw],
                            start=True,
                            stop=False,
                        )
                        psums.append(ps)
                    for (j0, jw), ps in zip(JCHUNKS, psums):
                        nc.tensor.matmul(
                            ps[:],
                            w2_sb[:],
                            h_blocks[ib + 1][: K - 1, j0 : j0 + jw],
                            start=False,
                            stop=True,
                        )
                    for (j0, jw), ps in zip(JCHUNKS, psums):
                        nc.scalar.copy(out_t[:, j0 : j0 + jw], ps[:])
                    nc.scalar.dma_start(out[ib * 128 : (ib + 1) * 128, :], out_t[:])

                if interleave:
                    # b0, b1, mm0, b2, mm1, b3, mm2, b4, mm3
                    emit_block(0)
                    emit_block(1)
                    for ib in range(N_OB):
                        if ib + 2 < N_BLK:
                            emit_block(ib + 2)
                        emit_mm_group(ib)
                else:
                    for b in range(N_BLK):
                        emit_block(b)
                    for ib in range(N_OB):
                        emit_mm_group(ib)

    nc.compile()
    return nc


def get_program(use_accum=False, repeat=1, gps_ops=0, interleave=True):
    key = (use_accum, repeat, gps_ops, interleave)
    if key not in _PROGRAM_CACHE:
        _PROGRAM_CACHE[key] = build_program(use_accum, repeat, gps_ops, interleave)
    return _PROGRAM_CACHE[key]


def make_in_maps(input_image, som_matrix, som_running_variances):
    img = np.ascontiguousarray(np.asarray(input_image, dtype=np.float32))
    som = np.ascontiguousarray(np.asarray(som_matrix, dtype=np.float32))
    var = np.ascontiguousarray(np.asarray(som_running_variances, dtype=np.float32))
    w1 = np.ascontiguousarray(_band_w1())
    w2 = np.ascontiguousarray(_band_w2())
    in_maps = []
    for c in range(N_CORES):
        s = STARTS[c]
        # slab-local row i is global row s+i -> kern row img[(s+i) % K]
        negkern = np.ascontiguousarray(
            (-np.tile(np.roll(img, -(s % K), axis=0), (128 // K, HH // K))).astype(
                np.float32
            )
        )
        in_maps.append(
            {
                "som": np.ascontiguousarray(som[s : s + DIST_ROWS]),
                "var": np.ascontiguousarray(var[s : s + DIST_ROWS]),
                "nkern": negkern,
                "w1": w1,
                "w2": w2,
            }
        )
    return in_maps


def assemble(results):
    out_full = np.empty((OUT, OUT), np.float32)
    for c in range(N_CORES):
        lo = STARTS[c]
        hi = STARTS[c + 1] if c < N_CORES - 1 else OUT
        out_full[lo:hi] = results[c]["out"][: hi - lo]
    return out_full


def kernel(input_image, som_matrix, som_running_variances):
    nc = get_program()
    in_maps = make_in_maps(input_image, som_matrix, som_running_variances)
    res = run_bass_kernel_spmd(nc, in_maps, core_ids=list(range(N_CORES)))
    return assemble(res.results)



# revision 2
# speedup vs baseline: 1.5454x; 1.5454x over previous
"""Trainium2 Bass kernel v2: low-precision HBM traffic + engine rebalance.

Reference computation (shapes hardcoded):
    kernel = tile(input_image[32,32] -> [4096,4096])
    dist   = (kernel - som_matrix)^2 / (som_running_variances + 1e-8)
    out    = 32x32 valid box-sum of dist -> [4065, 4065]

vs baseline: som travels as bf16 and w = 1/(var+eps) as bf16 or fp8-e4m3
(host converts; fp8 is cast up to bf16 inside the SDMA datapath on load),
the output travels as bf16 (host casts back to f32). The elementwise
chain runs in bf16 (DVE 2x perf mode), with a configurable number of
add/mul blocks offloaded to the Pool engine, square on ACT or DVE, the
horizontal 32-window box-sum as a DVE tensor_tensor_scan (Pool rejected
by walrus codegen), the vertical 32-row sum as banded bf16 matmuls on
PE, and the PSUM drain split between ACT and DVE per column chunk.
"""
import numpy as np
import ml_dtypes

import concourse.bass as bass
import concourse.mybir as mybir
import concourse.tile as tile
from concourse import bacc
from concourse.bass_utils import run_bass_kernel_spmd

K = 32
HH = 4096
OUT = HH - K + 1  # 4065
N_CORES = 8
OUT_ROWS = 512
DIST_ROWS = OUT_ROWS + K - 1  # 543
STARTS = [round(c * (OUT - OUT_ROWS) / (N_CORES - 1)) for c in range(N_CORES)]

PB = [128, 128, 128, 128, DIST_ROWS - 512]  # partition rows per block
RB = [0, 128, 256, 384, 512]
N_BLK = len(PB)
N_OB = 4  # output row-blocks of 128

F32 = mybir.dt.float32
BF16 = mybir.dt.bfloat16
FP8 = mybir.dt.float8e4

JCHUNKS = [(j, min(512, OUT - j)) for j in range(0, OUT, 512)]

_PROGRAM_CACHE = {}

DEFAULT_CFG = dict(
    w_dt="fp8",  # 'bf16' | 'fp8' : HBM dtype of w = 1/(var+eps)
    w_queue="swdge",  # 'hwdge' (scalar ring; bf16 only) | 'swdge' (gpsimd)
    add_pool=3,  # how many of the 5 add blocks run on Pool
    mul_pool=0,  # how many of the 5 w-mul blocks run on Pool
    sq_eng="act",  # 'act' | 'dve'
    drain_dve=1,  # of the 8 PSUM chunks per group, how many drain on DVE
    psum_bf16=False,  # matmul writes bf16 PSUM (halves drain read cost)
    drain_tiles=2,  # 8: one PSUM tile+copy per 512-chunk; 2: two big tiles
    out_dt="bf16",  # 'bf16' | 'fp8_offset' (store val-OUT_OFFSET as e4m3)
)

# box-sums of 1024 uniform-ish terms concentrate tightly around ~410 for any
# seed; e4m3 covers offset residuals up to +-448, actual span is ~+-110
OUT_OFFSET = 420.0


def bytes_per_iter(cfg=None):
    cfg = {**DEFAULT_CFG, **(cfg or {})}
    wb = 1 if cfg["w_dt"] == "fp8" else 2
    ob = 1 if cfg["out_dt"] == "fp8_offset" else 2
    return DIST_ROWS * HH * 2 + DIST_ROWS * HH * wb + OUT_ROWS * OUT * ob


def _band_w1():
    k = np.arange(128)[:, None]
    m = np.arange(128)[None, :]
    return ((m <= k) & (k <= m + K - 1)).astype(ml_dtypes.bfloat16)


def _band_w2():
    kk = np.arange(K - 1)[:, None]
    m = np.arange(128)[None, :]
    return (m >= kk + 128 - (K - 1)).astype(ml_dtypes.bfloat16)


def build_program(repeat=1, **cfg_kw):
    cfg = {**DEFAULT_CFG, **cfg_kw}
    w_hbm_dt = FP8 if cfg["w_dt"] == "fp8" else BF16
    nc = bacc.Bacc("TRN2", target_bir_lowering=False, debug=False)
    som = nc.dram_tensor("som", [DIST_ROWS, HH], BF16, kind="ExternalInput").ap()
    wdr = nc.dram_tensor("w", [DIST_ROWS, HH], w_hbm_dt, kind="ExternalInput").ap()
    nkern = nc.dram_tensor("nkern", [128, HH], BF16, kind="ExternalInput").ap()
    w1d = nc.dram_tensor("w1", [128, 128], BF16, kind="ExternalInput").ap()
    w2d = nc.dram_tensor("w2", [K - 1, 128], BF16, kind="ExternalInput").ap()
    out_hbm_dt = FP8 if cfg["out_dt"] == "fp8_offset" else BF16
    out = nc.dram_tensor(
        "out", [OUT_ROWS, OUT], out_hbm_dt, kind="ExternalOutput"
    ).ap()

    with tile.TileContext(nc) as tc:
        with (
            tc.tile_pool(name="const", bufs=1) as constp,
            tc.tile_pool(name="som", bufs=3) as somp,
            tc.tile_pool(name="w", bufs=2) as wp,
            tc.tile_pool(name="d", bufs=2) as dp,
            tc.tile_pool(name="h0", bufs=2) as h0p,
            tc.tile_pool(name="h", bufs=3) as hp,
            tc.tile_pool(name="outp", bufs=2) as outp,
            tc.tile_pool(
                name="psum",
                bufs=(1 if cfg["drain_tiles"] == 2 else 8),
                space="PSUM",
            ) as psump,
        ):
            nkern_sb = constp.tile([128, HH], BF16)
            nc.sync.dma_start(nkern_sb[:], nkern[:, :])
            w1_sb = constp.tile([128, 128], BF16)
            nc.sync.dma_start(w1_sb[:], w1d[:, :])
            w2_sb = constp.tile([K - 1, 128], BF16)
            nc.sync.dma_start(w2_sb[:], w2d[:, :])

            for _ in range(repeat):
                h_blocks = []

                def emit_block(b):
                    p = PB[b]
                    rows = slice(RB[b], RB[b] + p)
                    som_t = somp.tile([p, HH], BF16)
                    nc.sync.dma_start(som_t[:], som[rows, :])
                    # diff = som + (-kern), in place over som (bf16 TT, 2x)
                    add_eng = nc.gpsimd if b < cfg["add_pool"] else nc.vector
                    add_eng.tensor_add(som_t[:], som_t[:], nkern_sb[:p, :])
                    # w tile: bf16 in SBUF (fp8 HBM rides the SWDGE cast)
                    w_t = wp.tile([p, HH], BF16)
                    if cfg["w_queue"] == "hwdge" and cfg["w_dt"] == "bf16":
                        nc.scalar.dma_start(w_t[:], wdr[rows, :])
                    else:
                        nc.gpsimd.dma_start(w_t[:], wdr[rows, :])
                    # sq = diff^2
                    sq_t = dp.tile([p, HH], BF16)
                    if cfg["sq_eng"] == "act":
                        nc.scalar.activation(
                            sq_t[:], som_t[:], mybir.ActivationFunctionType.Square
                        )
                    else:
                        nc.vector.tensor_mul(sq_t[:], som_t[:], som_t[:])
                    # d = sq * w (bf16 2x)
                    mul_eng = nc.gpsimd if b < cfg["mul_pool"] else nc.vector
                    d_t = dp.tile([p, HH], BF16)
                    mul_eng.tensor_mul(d_t[:], sq_t[:], w_t[:])
                    # sliding 32-wide window sum in ONE scan pass:
                    #   h[0] = sum(d[0:32]);  h[j] = h[j-1] + d[j+31] - d[j-1]
                    h_t = hp.tile([p, OUT], BF16)
                    h0 = h0p.tile([p, 1], F32)
                    nc.vector.tensor_reduce(
                        h0[:], d_t[:, 0:K], mybir.AxisListType.X, mybir.AluOpType.add
                    )
                    nc.vector.tensor_copy(h_t[:, 0:1], h0[:])
                    nc.vector.tensor_tensor_scan(
                        h_t[:, 1:OUT],
                        d_t[:, K:HH],
                        d_t[:, 0 : OUT - 1],
                        initial=h0[:],
                        op0=mybir.AluOpType.add,
                        op1=mybir.AluOpType.subtract,
                    )
                    h_blocks.append(h_t)

                def emit_mm_group(ib):
                    out_t = outp.tile([128, OUT], out_hbm_dt)
                    ps_dt = BF16 if cfg["psum_bf16"] else F32

                    def drain(dst, ps, eng=nc.scalar):
                        if cfg["out_dt"] == "fp8_offset":
                            eng.activation(
                                dst,
                                ps,
                                mybir.ActivationFunctionType.Copy,
                                bias=-OUT_OFFSET,
                            )
                        else:
                            eng.copy(dst, ps)
                    if cfg["drain_tiles"] == 2:
                        # two big PSUM tiles; matmuls write 512-col slices
                        # (one bank each), drained with two large ACT copies
                        psA = psump.tile([128, 2048], ps_dt)
                        psB = psump.tile([128, OUT - 2048], ps_dt)

                        def chunk_ap(j0, jw):
                            return (
                                psA[:, j0 : j0 + jw]
                                if j0 < 2048
                                else psB[:, j0 - 2048 : j0 - 2048 + jw]
                            )

                        for j0, jw in JCHUNKS:
                            nc.tensor.matmul(
                                chunk_ap(j0, jw),
                                w1_sb[:],
                                h_blocks[ib][:, j0 : j0 + jw],
                                start=True,
                                stop=False,
                            )
                        for j0, jw in JCHUNKS:
                            nc.tensor.matmul(
                                chunk_ap(j0, jw),
                                w2_sb[:],
                                h_blocks[ib + 1][: K - 1, j0 : j0 + jw],
                                start=False,
                                stop=True,
                            )
                        drain(out_t[:, 0:2048], psA[:])
                        drain(out_t[:, 2048:OUT], psB[:])
                    else:
                        psums = []
                        for j0, jw in JCHUNKS:
                            ps = psump.tile([128, jw], ps_dt)
                            nc.tensor.matmul(
                                ps[:],
                                w1_sb[:],
                                h_blocks[ib][:, j0 : j0 + jw],
                                start=True,
                                stop=False,
                            )
                            psums.append(ps)
                        for (j0, jw), ps in zip(JCHUNKS, psums):
                            nc.tensor.matmul(
                                ps[:],
                                w2_sb[:],
                                h_blocks[ib + 1][: K - 1, j0 : j0 + jw],
                                start=False,
                                stop=True,
                            )
                        for ci, ((j0, jw), ps) in enumerate(zip(JCHUNKS, psums)):
                            if ci < cfg["drain_dve"] and cfg["out_dt"] == "bf16":
                                nc.vector.tensor_copy(out_t[:, j0 : j0 + jw], ps[:])
                            else:
                                drain(out_t[:, j0 : j0 + jw], ps[:])
                    nc.scalar.dma_start(out[ib * 128 : (ib + 1) * 128, :], out_t[:])

                # b0, b1, mm0, b2, mm1, b3, mm2, b4, mm3
                emit_block(0)
                emit_block(1)
                for ib in range(N_OB):
                    if ib + 2 < N_BLK:
                        emit_block(ib + 2)
                    emit_mm_group(ib)

    nc.compile()
    return nc


def get_program(repeat=1, **cfg_kw):
    key = (repeat, tuple(sorted({**DEFAULT_CFG, **cfg_kw}.items())))
    if key not in _PROGRAM_CACHE:
        _PROGRAM_CACHE[key] = build_program(repeat, **cfg_kw)
    return _PROGRAM_CACHE[key]


def make_in_maps(input_image, som_matrix, som_running_variances, w_dt=None):
    w_dt = w_dt or DEFAULT_CFG["w_dt"]
    np_w_dt = ml_dtypes.float8_e4m3 if w_dt == "fp8" else ml_dtypes.bfloat16
    img = np.asarray(input_image, dtype=np.float32)
    som = np.asarray(som_matrix, dtype=np.float32).astype(ml_dtypes.bfloat16)
    w = (
        1.0 / (np.asarray(som_running_variances, dtype=np.float32) + 1e-8)
    ).astype(np_w_dt)
    w1 = np.ascontiguousarray(_band_w1())
    w2 = np.ascontiguousarray(_band_w2())
    in_maps = []
    for c in range(N_CORES):
        s = STARTS[c]
        # slab-local row i is global row s+i -> kern row img[(s+i) % K]
        negkern = np.ascontiguousarray(
            (-np.tile(np.roll(img, -(s % K), axis=0), (128 // K, HH // K))).astype(
                ml_dtypes.bfloat16
            )
        )
        in_maps.append(
            {
                "som": np.ascontiguousarray(som[s : s + DIST_ROWS]),
                "w": np.ascontiguousarray(w[s : s + DIST_ROWS]),
                "nkern": negkern,
                "w1": w1,
                "w2": w2,
            }
        )
    return in_maps


def assemble(results, out_dt=None):
    out_dt = out_dt or DEFAULT_CFG["out_dt"]
    off = OUT_OFFSET if out_dt == "fp8_offset" else 0.0
    out_full = np.empty((OUT, OUT), np.float32)
    for c in range(N_CORES):
        lo = STARTS[c]
        hi = STARTS[c + 1] if c < N_CORES - 1 else OUT
        out_full[lo:hi] = results[c]["out"][: hi - lo].astype(np.float32) + off
    return out_full


def kernel(input_image, som_matrix, som_running_variances):
    nc = get_program()
    in_maps = make_in_maps(input_image, som_matrix, som_running_variances)
    res = run_bass_kernel_spmd(nc, in_maps, core_ids=list(range(N_CORES)))
    return assemble(res.results)


# revision 4
# speedup vs baseline: 2.5861x; 1.6734x over previous
"""Trainium2 Bass kernel v2: low-precision HBM traffic + engine rebalance.

Reference computation (shapes hardcoded):
    kernel = tile(input_image[32,32] -> [4096,4096])
    dist   = (kernel - som_matrix)^2 / (som_running_variances + 1e-8)
    out    = 32x32 valid box-sum of dist -> [4065, 4065]

vs baseline: som travels as bf16 and w = 1/(var+eps) as bf16 or fp8-e4m3
(host converts; fp8 is cast up to bf16 inside the SDMA datapath on load),
the output travels as bf16 (host casts back to f32). The elementwise
chain runs in bf16 (DVE 2x perf mode), with a configurable number of
add/mul blocks offloaded to the Pool engine, square on ACT or DVE, the
horizontal 32-window box-sum as a DVE tensor_tensor_scan (Pool rejected
by walrus codegen), the vertical 32-row sum as banded bf16 matmuls on
PE, and the PSUM drain split between ACT and DVE per column chunk.
"""
import numpy as np
import ml_dtypes

import concourse.bass as bass
import concourse.mybir as mybir
import concourse.tile as tile
from concourse import bacc
from concourse.bass_utils import run_bass_kernel_spmd

K = 32
HH = 4096
OUT = HH - K + 1  # 4065
N_CORES = 8
OUT_ROWS = 512
DIST_ROWS = OUT_ROWS + K - 1  # 543
STARTS = [round(c * (OUT - OUT_ROWS) / (N_CORES - 1)) for c in range(N_CORES)]

PB = [128, 128, 128, 128, DIST_ROWS - 512]  # partition rows per block
RB = [0, 128, 256, 384, 512]
N_BLK = len(PB)
N_OB = 4  # output row-blocks of 128

F32 = mybir.dt.float32
BF16 = mybir.dt.bfloat16
FP8 = mybir.dt.float8e4

JCHUNKS = [(j, min(512, OUT - j)) for j in range(0, OUT, 512)]

_PROGRAM_CACHE = {}

DEFAULT_CFG = dict(
    w_dt="fp8",  # 'bf16' | 'fp8' : HBM dtype of w = 1/(var+eps)
    w_queue="swdge",  # 'hwdge' (scalar ring; bf16 only) | 'swdge' (gpsimd)
    add_pool=0,  # Pool TT ops contend with DVE on the shared SBUF port
    mul_pool=0,  # pair (measured super-additive) - keep Pool to DGE only
    sq_eng="act",  # 'act' | 'dve'
    drain_dve=1,  # of the 8 PSUM chunks per group, how many drain on DVE
    psum_bf16=False,  # matmul writes bf16 PSUM (halves drain read cost)
    drain_tiles=2,  # 8: one PSUM tile+copy per 512-chunk; 2: two big tiles
    out_dt="bf16",  # 'bf16' | 'fp8_offset' (store val-OUT_OFFSET as e4m3)
    pair_loads=False,  # (unused) load 2 row-blocks per DMA
    staged=False,  # software-pipeline emission (measured: no gain)
)

# box-sums of 1024 uniform-ish terms concentrate tightly around ~410 for any
# seed; e4m3 covers offset residuals up to +-448, actual span is ~+-110
OUT_OFFSET = 420.0


def bytes_per_iter(cfg=None):
    cfg = {**DEFAULT_CFG, **(cfg or {})}
    wb = 1 if cfg["w_dt"] == "fp8" else 2
    ob = 1 if cfg["out_dt"] == "fp8_offset" else 2
    return DIST_ROWS * HH * 2 + DIST_ROWS * HH * wb + OUT_ROWS * OUT * ob


def _band_w1():
    k = np.arange(128)[:, None]
    m = np.arange(128)[None, :]
    return ((m <= k) & (k <= m + K - 1)).astype(ml_dtypes.bfloat16)


def _band_w2():
    kk = np.arange(K - 1)[:, None]
    m = np.arange(128)[None, :]
    return (m >= kk + 128 - (K - 1)).astype(ml_dtypes.bfloat16)


def build_program(repeat=1, **cfg_kw):
    cfg = {**DEFAULT_CFG, **cfg_kw}
    w_hbm_dt = FP8 if cfg["w_dt"] == "fp8" else BF16
    nc = bacc.Bacc("TRN2", target_bir_lowering=False, debug=False)
    som = nc.dram_tensor("som", [DIST_ROWS, HH], BF16, kind="ExternalInput").ap()
    wdr = nc.dram_tensor("w", [DIST_ROWS, HH], w_hbm_dt, kind="ExternalInput").ap()
    nkern = nc.dram_tensor("nkern", [128, HH], BF16, kind="ExternalInput").ap()
    w1d = nc.dram_tensor("w1", [128, 128], BF16, kind="ExternalInput").ap()
    w2d = nc.dram_tensor("w2", [K - 1, 128], BF16, kind="ExternalInput").ap()
    out_hbm_dt = FP8 if cfg["out_dt"] == "fp8_offset" else BF16
    out = nc.dram_tensor(
        "out", [OUT_ROWS, OUT], out_hbm_dt, kind="ExternalOutput"
    ).ap()

    with tile.TileContext(nc) as tc:
        with (
            tc.tile_pool(name="const", bufs=1) as constp,
            tc.tile_pool(name="som", bufs=3) as somp,
            tc.tile_pool(name="w", bufs=(3 if cfg["staged"] else 2)) as wp,
            tc.tile_pool(name="d", bufs=(4 if cfg["staged"] else 2)) as dp,
            tc.tile_pool(name="h0", bufs=2) as h0p,
            tc.tile_pool(name="h", bufs=3) as hp,
            tc.tile_pool(name="outp", bufs=2) as outp,
            tc.tile_pool(
                name="psum",
                bufs=(1 if cfg["drain_tiles"] == 2 else 8),
                space="PSUM",
            ) as psump,
        ):
            nkern_sb = constp.tile([128, HH], BF16)
            nc.sync.dma_start(nkern_sb[:], nkern[:, :])
            w1_sb = constp.tile([128, 128], BF16)
            nc.sync.dma_start(w1_sb[:], w1d[:, :])
            w2_sb = constp.tile([K - 1, 128], BF16)
            nc.sync.dma_start(w2_sb[:], w2d[:, :])

            for _ in range(repeat):
                h_blocks = []

                stageA_out = {}

                def emit_stageA(b):
                    p = PB[b]
                    rows = slice(RB[b], RB[b] + p)
                    som_t = somp.tile([p, HH], BF16)
                    nc.sync.dma_start(som_t[:], som[rows, :])
                    # diff = som + (-kern), in place over som (bf16 TT, 2x)
                    add_eng = nc.gpsimd if b < cfg["add_pool"] else nc.vector
                    add_eng.tensor_add(som_t[:], som_t[:], nkern_sb[:p, :])
                    # w tile: bf16 in SBUF (fp8 HBM rides the SWDGE cast)
                    w_t = wp.tile([p, HH], BF16)
                    if cfg["w_queue"] == "sync" and cfg["w_dt"] == "bf16":
                        nc.sync.dma_start(w_t[:], wdr[rows, :])
                    elif cfg["w_queue"] == "hwdge" and cfg["w_dt"] == "bf16":
                        nc.scalar.dma_start(w_t[:], wdr[rows, :])
                    else:
                        nc.gpsimd.dma_start(w_t[:], wdr[rows, :])
                    # sq = diff^2
                    sq_t = dp.tile([p, HH], BF16)
                    if cfg["sq_eng"] == "act":
                        nc.scalar.activation(
                            sq_t[:], som_t[:], mybir.ActivationFunctionType.Square
                        )
                    else:
                        nc.vector.tensor_mul(sq_t[:], som_t[:], som_t[:])
                    stageA_out[b] = (sq_t, w_t)

                def emit_stageB(b):
                    p = PB[b]
                    sq_t, w_t = stageA_out[b]
                    # d = sq * w (bf16 2x)
                    mul_eng = nc.gpsimd if b < cfg["mul_pool"] else nc.vector
                    d_t = dp.tile([p, HH], BF16)
                    mul_eng.tensor_mul(d_t[:], sq_t[:], w_t[:])
                    # sliding 32-wide window sum in ONE scan pass:
                    #   h[0] = sum(d[0:32]);  h[j] = h[j-1] + d[j+31] - d[j-1]
                    h_t = hp.tile([p, OUT], BF16)
                    h0 = h0p.tile([p, 1], F32)
                    nc.vector.tensor_reduce(
                        h0[:], d_t[:, 0:K], mybir.AxisListType.X, mybir.AluOpType.add
                    )
                    nc.vector.tensor_copy(h_t[:, 0:1], h0[:])
                    nc.vector.tensor_tensor_scan(
                        h_t[:, 1:OUT],
                        d_t[:, K:HH],
                        d_t[:, 0 : OUT - 1],
                        initial=h0[:],
                        op0=mybir.AluOpType.add,
                        op1=mybir.AluOpType.subtract,
                    )
                    h_blocks.append(h_t)

                def emit_block(b):
                    emit_stageA(b)
                    emit_stageB(b)

                def emit_mm_group(ib):
                    out_t = outp.tile([128, OUT], out_hbm_dt)
                    ps_dt = BF16 if cfg["psum_bf16"] else F32

                    def drain(dst, ps, eng=nc.scalar):
                        if cfg["out_dt"] == "fp8_offset":
                            eng.activation(
                                dst,
                                ps,
                                mybir.ActivationFunctionType.Copy,
                                bias=-OUT_OFFSET,
                            )
                        else:
                            eng.copy(dst, ps)
                    if cfg["drain_tiles"] == 2:
                        # two big PSUM tiles; matmuls write 512-col slices
                        # (one bank each), drained with two large ACT copies
                        psA = psump.tile([128, 2048], ps_dt)
                        psB = psump.tile([128, OUT - 2048], ps_dt)

                        def chunk_ap(j0, jw):
                            return (
                                psA[:, j0 : j0 + jw]
                                if j0 < 2048
                                else psB[:, j0 - 2048 : j0 - 2048 + jw]
                            )

                        for j0, jw in JCHUNKS:
                            nc.tensor.matmul(
                                chunk_ap(j0, jw),
                                w1_sb[:],
                                h_blocks[ib][:, j0 : j0 + jw],
                                start=True,
                                stop=False,
                            )
                        for j0, jw in JCHUNKS:
                            nc.tensor.matmul(
                                chunk_ap(j0, jw),
                                w2_sb[:],
                                h_blocks[ib + 1][: K - 1, j0 : j0 + jw],
                                start=False,
                                stop=True,
                            )
                        drain(out_t[:, 0:2048], psA[:])
                        drain(out_t[:, 2048:OUT], psB[:])
                    else:
                        psums = []
                        for j0, jw in JCHUNKS:
                            ps = psump.tile([128, jw], ps_dt)
                            nc.tensor.matmul(
                                ps[:],
                                w1_sb[:],
                                h_blocks[ib][:, j0 : j0 + jw],
                                start=True,
                                stop=False,
                            )
                            psums.append(ps)
                        for (j0, jw), ps in zip(JCHUNKS, psums):
                            nc.tensor.matmul(
                                ps[:],
                                w2_sb[:],
                                h_blocks[ib + 1][: K - 1, j0 : j0 + jw],
                                start=False,
                                stop=True,
                            )
                        for ci, ((j0, jw), ps) in enumerate(zip(JCHUNKS, psums)):
                            if ci < cfg["drain_dve"] and cfg["out_dt"] == "bf16":
                                nc.vector.tensor_copy(out_t[:, j0 : j0 + jw], ps[:])
                            else:
                                drain(out_t[:, j0 : j0 + jw], ps[:])
                    nc.scalar.dma_start(out[ib * 128 : (ib + 1) * 128, :], out_t[:])

                if cfg["staged"]:
                    # A0 A1 B0 A2 B1 mm0 A3 B2 mm1 A4 B3 mm2 B4 mm3
                    emit_stageA(0)
                    emit_stageA(1)
                    emit_stageB(0)
                    emit_stageA(2)
                    emit_stageB(1)
                    emit_mm_group(0)
                    emit_stageA(3)
                    emit_stageB(2)
                    emit_mm_group(1)
                    emit_stageA(4)
                    emit_stageB(3)
                    emit_mm_group(2)
                    emit_stageB(4)
                    emit_mm_group(3)
                else:
                    # b0, b1, mm0, b2, mm1, b3, mm2, b4, mm3
                    emit_block(0)
                    emit_block(1)
                    for ib in range(N_OB):
                        if ib + 2 < N_BLK:
                            emit_block(ib + 2)
                        emit_mm_group(ib)

    nc.compile()
    return nc


def get_program(repeat=1, **cfg_kw):
    key = (repeat, tuple(sorted({**DEFAULT_CFG, **cfg_kw}.items())))
    if key not in _PROGRAM_CACHE:
        _PROGRAM_CACHE[key] = build_program(repeat, **cfg_kw)
    return _PROGRAM_CACHE[key]


def make_in_maps(input_image, som_matrix, som_running_variances, w_dt=None):
    w_dt = w_dt or DEFAULT_CFG["w_dt"]
    np_w_dt = ml_dtypes.float8_e4m3 if w_dt == "fp8" else ml_dtypes.bfloat16
    img = np.asarray(input_image, dtype=np.float32)
    som = np.asarray(som_matrix, dtype=np.float32).astype(ml_dtypes.bfloat16)
    w = (
        1.0 / (np.asarray(som_running_variances, dtype=np.float32) + 1e-8)
    ).astype(np_w_dt)
    w1 = np.ascontiguousarray(_band_w1())
    w2 = np.ascontiguousarray(_band_w2())
    in_maps = []
    for c in range(N_CORES):
        s = STARTS[c]
        # slab-local row i is global row s+i -> kern row img[(s+i) % K]
        negkern = np.ascontiguousarray(
            (-np.tile(np.roll(img, -(s % K), axis=0), (128 // K, HH // K))).astype(
                ml_dtypes.bfloat16
            )
        )
        in_maps.append(
            {
                "som": np.ascontiguousarray(som[s : s + DIST_ROWS]),
                "w": np.ascontiguousarray(w[s : s + DIST_ROWS]),
                "nkern": negkern,
                "w1": w1,
                "w2": w2,
            }
        )
    return in_maps


def assemble(results, out_dt=None):
    out_dt = out_dt or DEFAULT_CFG["out_dt"]
    off = OUT_OFFSET if out_dt == "fp8_offset" else 0.0
    out_full = np.empty((OUT, OUT), np.float32)
    for c in range(N_CORES):
        lo = STARTS[c]
        hi = STARTS[c + 1] if c < N_CORES - 1 else OUT
        out_full[lo:hi] = results[c]["out"][: hi - lo].astype(np.float32) + off
    return out_full


def kernel(input_image, som_matrix, som_running_variances):
    nc = get_program()
    in_maps = make_in_maps(input_image, som_matrix, som_running_variances)
    res = run_bass_kernel_spmd(nc, in_maps, core_ids=list(range(N_CORES)))
    return assemble(res.results)
